# revision 46
# baseline (speedup 1.0000x reference)
"""Trainium2 Bass kernel for nn_AttentionLayerBase (relative-position banded attention).

Sharding: 16 heads over 8 cores (2 heads/core, tensor parallel). All matmuls in
bf16 (inputs host-cast), f32 PSUM accumulation. Per core:
  - Q^T, K^T, V (t-major) and R^T projected from X^T; per-head stationaries are
    zero-padded to full K=128 contraction (the HAM clock gate tracks PE *array
    activity*, so partial-height matmuls would pin the clock at 1.2 GHz)
  - per q-tile: P_rev band = R @ reversed(b_nd); the band is EXPONENTIATED
    during its PSUM eviction (exp(S+B) = exp(S) * exp(B)), then a
    diagonal-stride SBUF->SBUF DMA materializes expB[p,j] = exp(P[p, p-j])
    (Toeplitz shear, no HBM round trip); causal mask fills 0 in exp domain
  - S = Q K^T/sqrt(d) -> exp on ACT; one DVE scalar_tensor_tensor fuses the
    exp(S)*expB multiply with the softmax row-sum accumulation
  - A^T tiles via PE transpose (8 per PSUM bank); AV accumulated with
    V-stationary matmuls, both heads col-packed into one PSUM tile
  - output projection per head (zero-padded context stationaries); softmax
    1/rowsum applied per head at PSUM eviction; loop software-pipelined
    (extra(ti+1) prepared during tile ti, outproj(ti) deferred into ti+1)
Partial projections written as bf16; host sums the 8 partials + b_proj in f32.
"""

import numpy as np
import ml_dtypes

import concourse.bass as bass
import concourse.mybir as mybir
import concourse.tile as tile
from concourse import bacc
from concourse.masks import make_identity

FP = mybir.dt.float32
BF = mybir.dt.bfloat16
BF_NP = ml_dtypes.bfloat16
N_HEADS = 16
N_CORES = 8
HPC = N_HEADS // N_CORES  # heads per core = 2
E = 1024
DH = 64            # head dim (qk and v)
FQ = HPC * DH      # per-core q/k/v feature cols = 128
NB = 10            # n_basis
FR = HPC * NB      # per-core r cols = 20
ML = 2048          # max_len of b_nd
SCALE = 1.0 / 8.0  # 1/sqrt(64)
NEG = -1e9


def build_nc(T):
    NT = T // 128
    NE = E // 128
    nc = bacc.Bacc("TRN2", target_bir_lowering=False, debug=False)

    xt_d = nc.dram_tensor("xt", [128, E // 128, T], BF, kind="ExternalInput")
    wq_d = nc.dram_tensor("wq", [128, E // 128, FQ], BF, kind="ExternalInput")
    wk_d = nc.dram_tensor("wk", [128, E // 128, FQ], BF, kind="ExternalInput")
    wv_d = nc.dram_tensor("wv", [128, E // 128, FQ], BF, kind="ExternalInput")
    wr_d = nc.dram_tensor("wr", [128, E // 128, FR], BF, kind="ExternalInput")
    wp_d = nc.dram_tensor("wp", [FQ, E], BF, kind="ExternalInput")
    bq_d = nc.dram_tensor("bq8", [FQ, 1], FP, kind="ExternalInput")
    br_d = nc.dram_tensor("br", [42, 1], FP, kind="ExternalInput")
    brev_d = nc.dram_tensor("brev", [128, ML], BF, kind="ExternalInput")
    out_d = nc.dram_tensor("out_partial", [T, E], BF, kind="ExternalOutput")

    with tile.TileContext(nc) as tc:
        with (
            tc.tile_pool(name="const", bufs=1) as const_pool,
            tc.tile_pool(name="big", bufs=1) as big_pool,
            tc.tile_pool(name="work", bufs=3) as work_pool,
            tc.tile_pool(name="small", bufs=5) as small_pool,
            tc.tile_pool(name="ps_s", bufs=3, space="PSUM") as ps_s,
            tc.tile_pool(name="ps_p", bufs=2, space="PSUM") as ps_p,
            tc.tile_pool(name="ps_t", bufs=2, space="PSUM") as ps_t,
            tc.tile_pool(name="ps_av", bufs=1, space="PSUM") as ps_av,
        ):
            # ---- constants / weights into SBUF ----
            ident = const_pool.tile([128, 128], BF)
            make_identity(nc, ident[:])

            xt_sb = big_pool.tile([128, NE, T], BF)
            wq_sb = const_pool.tile([128, NE, FQ], BF)
            wk_sb = const_pool.tile([128, NE, FQ], BF)
            wv_sb = const_pool.tile([128, NE, FQ], BF)
            wr_sb = const_pool.tile([128, NE, FR], BF)
            wp_sb = const_pool.tile([128, E], BF)
            bq_sb = const_pool.tile([FQ, 1], FP)
            br_sb = const_pool.tile([42, 1], FP)
            brev_sb = const_pool.tile([128, ML], BF)
            # weights first so the first projection matmuls fire as soon as
            # the first X chunks land; X interleaved across both HWDGE rings
            nc.sync.dma_start(wq_sb[:], wq_d[:])
            nc.scalar.dma_start(wk_sb[:], wk_d[:])
            nc.sync.dma_start(wr_sb[:], wr_d[:])
            nc.scalar.dma_start(bq_sb[:], bq_d[:])
            nc.scalar.dma_start(br_sb[:], br_d[:])
            # column-major X load: the projection c0-loop consumes columns
            # in order, so the first matmuls fire after ~1MB instead of 4MB
            for cq in range(0, T, 512):
                for e in range(NE):
                    eng = nc.sync if e % 2 == 0 else nc.scalar
                    eng.dma_start(xt_sb[:, e, cq:cq + 512], xt_d[:, e, cq:cq + 512])
            nc.sync.dma_start(wv_sb[:], wv_d[:])
            nc.scalar.dma_start(brev_sb[:], brev_d[:])
            nc.scalar.dma_start(wp_sb[:], wp_d[:])

            # ---- projections ----
            # Per-head stationaries zero-padded to full K=128 contraction: the
            # HAM clock gate tracks PE *array activity*, so half/partial-height
            # matmuls never register busy enough to unthrottle. Padding rows
            # multiply against the other head's rows times zero.
            qT_pad = [big_pool.tile([128, T], BF, name=f"qTp{h}") for h in range(HPC)]
            kT_sb = big_pool.tile([128, T], BF)
            rT_pad = [big_pool.tile([128, T], BF, name=f"rTp{h}") for h in range(HPC)]
            v_sb = big_pool.tile([128, NT, FQ], BF)    # t-major; head h cols 64h:64h+64
            nc.gpsimd.memset(qT_pad[0][64:128, :], 0.0)
            nc.gpsimd.memset(qT_pad[1][0:64, :], 0.0)
            nc.gpsimd.memset(rT_pad[0][:], 0.0)
            nc.gpsimd.memset(rT_pad[1][:], 0.0)

            for c0 in range(0, T, 512):
                ct = min(c0 + 512, T)
                qp = ps_s.tile([128, 512], FP, name="qp", tag="mm")
                for e in range(NE):
                    nc.tensor.matmul(qp[:, :ct - c0], wq_sb[:, e], xt_sb[:, e, c0:ct],
                                     start=(e == 0), stop=(e == NE - 1))
                nc.scalar.activation(qT_pad[0][0:64, c0:ct], qp[0:64, :ct - c0],
                                     mybir.ActivationFunctionType.Identity,
                                     bias=bq_sb[0:64, 0:1], scale=SCALE)
                nc.scalar.activation(qT_pad[1][64:128, c0:ct], qp[64:128, :ct - c0],
                                     mybir.ActivationFunctionType.Identity,
                                     bias=bq_sb[64:128, 0:1], scale=SCALE)
                kp = ps_s.tile([128, 512], FP, name="kp", tag="mm")
                for e in range(NE):
                    nc.tensor.matmul(kp[:, :ct - c0], wk_sb[:, e], xt_sb[:, e, c0:ct],
                                     start=(e == 0), stop=(e == NE - 1))
                nc.vector.tensor_copy(kT_sb[:, c0:ct], kp[:, :ct - c0])
                # R^T both heads in one psum tile, col groups 0 / 1
                rp = ps_p.tile([128, 512], FP, name="rp", tag="pp")
                for e in range(NE):
                    nc.tensor.matmul(rp[0:NB, :ct - c0], wr_sb[:, e, 0:NB],
                                     xt_sb[:, e, c0:ct],
                                     start=(e == 0), stop=(e == NE - 1))
                    nc.tensor.matmul(rp[32:32 + NB, :ct - c0], wr_sb[:, e, NB:2 * NB],
                                     xt_sb[:, e, c0:ct],
                                     start=(e == 0), stop=(e == NE - 1))
                nc.scalar.activation(rT_pad[0][0:NB, c0:ct], rp[0:NB, :ct - c0],
                                     mybir.ActivationFunctionType.Identity,
                                     bias=br_sb[0:NB, 0:1], scale=1.0)
                nc.scalar.activation(rT_pad[1][32:32 + NB, c0:ct], rp[32:32 + NB, :ct - c0],
                                     mybir.ActivationFunctionType.Identity,
                                     bias=br_sb[32:32 + NB, 0:1], scale=1.0)
            for ti in range(NT):
                vp = ps_s.tile([128, 512], FP, name="vp", tag="mm")[:, :FQ]
                for e in range(NE):
                    nc.tensor.matmul(vp[:], xt_sb[:, e, ti * 128:(ti + 1) * 128],
                                     wv_sb[:, e], start=(e == 0), stop=(e == NE - 1))
                nc.vector.tensor_copy(v_sb[:, ti], vp[:])

            # ---- attention: software-pipelined over q-tiles ----
            # prep(ti): P_rev band matmuls -> bf16 evict -> diagonal-shear DMA
            # -> causal mask, producing extra tiles for tile ti.
            def prep(ti):
                W = 128 * (ti + 1)
                pbands = [work_pool.tile([128, T + 128], BF, name=f"pband{h}")
                          for h in range(HPC)]
                for cs in range(0, W, 512):
                    ce = min(cs + 512, W)
                    pps = []
                    for h in range(HPC):
                        pp = ps_p.tile([128, 512], FP, name="pp", tag="pp")
                        nc.tensor.matmul(pp[:, :ce - cs],
                                         rT_pad[h][:, ti * 128:(ti + 1) * 128],
                                         brev_sb[:, ML - W + cs:ML - W + ce],
                                         start=True, stop=True)
                        pps.append(pp)
                    for h in range(HPC):
                        # exp(S + extra) = exp(S) * exp(extra): exponentiate the
                        # band during its PSUM eviction (same-cost ACT pass)
                        nc.scalar.activation(pbands[h][:, cs:ce], pps[h][:, :ce - cs],
                                             mybir.ActivationFunctionType.Exp)
                extras = []
                for h in range(HPC):
                    extra = work_pool.tile([128, T], BF, name=f"extra{h}")
                    # shear: extra[p, j] = pband[p, 127 - p + j]; stride is
                    # relative to the tile's flat row length (T + 128)
                    esrc = bass.AP(pbands[h][:].tensor, 127, [[T + 128 - 1, 128], [1, W]])
                    nc.sync.dma_start(extra[:, :W], esrc)
                    nc.gpsimd.affine_select(
                        out=extra[:, W - 128:W], in_=extra[:, W - 128:W],
                        compare_op=mybir.AluOpType.is_ge, fill=0.0,
                        base=0, channel_multiplier=1, pattern=[[-1, 128]])
                    extras.append(extra)
                return extras

            def outproj(ti, recips, outps):
                po_sb = work_pool.tile([128, E], BF, name="po")
                for n0 in range(0, E, 512):
                    ops = []
                    for h in range(HPC):
                        op = ps_t.tile([128, 512], FP, name=f"op{h}", tag="tp4")
                        nc.tensor.matmul(op[:], outps[h][:],
                                         wp_sb[:, n0:n0 + 512],
                                         start=True, stop=True)
                        ops.append(op)
                    if n0 == 0:
                        nc.scalar.mul(po_sb[:, n0:n0 + 512], ops[0][:], recips[0][:, 0:1])
                    else:
                        nc.vector.tensor_scalar_mul(po_sb[:, n0:n0 + 512], ops[0][:],
                                                    recips[0][:, 0:1])
                    nc.vector.scalar_tensor_tensor(
                        po_sb[:, n0:n0 + 512], ops[1][:], recips[1][:, 0:1],
                        po_sb[:, n0:n0 + 512],
                        mybir.AluOpType.mult, mybir.AluOpType.add)
                nc.sync.dma_start(out_d[ti * 128:(ti + 1) * 128, :], po_sb[:])

            extras = prep(0)
            prev = None  # (ti, recips) pending output projection
            for ti in range(NT):
                W = 128 * (ti + 1)
                # S = Q K^T/8 + extra, exp with fused row sums
                aexps = [work_pool.tile([128, T], BF, name=f"aexp{h}")
                         for h in range(HPC)]
                aexpSs = [work_pool.tile([128, T], BF, name=f"aexpS{h}")
                          for h in range(HPC)]
                sums4 = [small_pool.tile([128, T // 512 + 1], FP, name=f"sums4{h}")
                         for h in range(HPC)]
                nch = 0
                for cs in range(0, W, 512):
                    ce = min(cs + 512, W)
                    sps = []
                    for h in range(HPC):
                        sp = ps_s.tile([128, 512], FP, name="sp", tag="mm")
                        nc.tensor.matmul(sp[:, :ce - cs],
                                         qT_pad[h][:, ti * 128:(ti + 1) * 128],
                                         kT_sb[:, cs:ce],
                                         start=True, stop=True)
                        sps.append(sp)
                    for h in range(HPC):
                        nc.scalar.activation(aexpSs[h][:, cs:ce], sps[h][:, :ce - cs],
                                             mybir.ActivationFunctionType.Exp)
                        nc.vector.scalar_tensor_tensor(
                            aexps[h][:, cs:ce], aexpSs[h][:, cs:ce], 1.0,
                            extras[h][:, cs:ce],
                            mybir.AluOpType.bypass, mybir.AluOpType.mult,
                            accum_out=sums4[h][:, nch:nch + 1])
                    nch += 1

                # deferred output projection of the previous tile (its avp
                # eviction is long done -> no PE FIFO stall)
                if prev is not None:
                    outproj(*prev)

                # prepare extra for the NEXT tile while exp/AV of this one runs
                if ti + 1 < NT:
                    next_extras = prep(ti + 1)

                recips = []
                for h in range(HPC):
                    sums1 = small_pool.tile([128, 1], FP, name="sums1")
                    if nch > 1:
                        nc.vector.tensor_reduce(sums1[:], sums4[h][:, :nch],
                                                axis=mybir.AxisListType.X,
                                                op=mybir.AluOpType.add)
                    else:
                        nc.vector.tensor_copy(sums1[:], sums4[h][:, 0:1])
                    recip = small_pool.tile([128, 1], FP, name=f"recip{h}")
                    nc.vector.reciprocal(recip[:], sums1[:])
                    recips.append(recip)

                # A^T tiles (batched transposes) + AV accumulate; both heads
                # col-packed into one PSUM tile [128 (h,d), 128 q]
                avp = ps_av.tile([128, 128], FP, name="avp")
                for j0 in range(0, ti + 1, 8):
                    j1 = min(j0 + 8, ti + 1)
                    aT4s = []
                    for h in range(HPC):
                        tp4 = ps_t.tile([128, 1024], BF, name="tp4")
                        for c in range(j1 - j0):
                            j = j0 + c
                            nc.tensor.matmul(tp4[:, c * 128:(c + 1) * 128],
                                             aexps[h][:, j * 128:(j + 1) * 128],
                                             ident[:], is_transpose=True)
                        aT4 = small_pool.tile([128, 1024], BF, name="aT4")
                        nc.vector.tensor_copy(aT4[:, :(j1 - j0) * 128],
                                              tp4[:, :(j1 - j0) * 128])
                        aT4s.append(aT4)
                    for c in range(j1 - j0):
                        j = j0 + c
                        for h in range(HPC):
                            nc.tensor.matmul(avp[64 * h:64 * h + DH, :],
                                             v_sb[:, j, DH * h:DH * (h + 1)],
                                             aT4s[h][:, c * 128:(c + 1) * 128],
                                             start=(j == 0), stop=(j == ti))
                outps = [small_pool.tile([128, 128], BF, name=f"outp{h}")
                         for h in range(HPC)]
                nc.gpsimd.memset(outps[0][64:128, :], 0.0)
                nc.gpsimd.memset(outps[1][0:64, :], 0.0)
                nc.vector.tensor_copy(outps[0][0:64, :], avp[0:64, :])
                nc.vector.tensor_copy(outps[1][64:128, :], avp[64:128, :])
                prev = (ti, recips, outps)
                if ti + 1 < NT:
                    extras = next_extras
            outproj(*prev)

    nc.compile()
    return nc


def make_in_maps(inputs, T):
    X = np.asarray(inputs["X_bte"], np.float32)[0]  # (T, E)
    xt = np.ascontiguousarray(
        X.T.reshape(E // 128, 128, -1).transpose(1, 0, 2)).astype(BF_NP)
    rearr = lambda w: np.ascontiguousarray(
        w.reshape(E // 128, 128, -1).transpose(1, 0, 2)).astype(BF_NP)
    brev_1 = np.asarray(inputs["b_nd"], np.float32)[:, ::-1].astype(BF_NP)
    brev = np.zeros((128, ML), BF_NP)
    brev[0:NB] = brev_1
    brev[32:32 + NB] = brev_1
    W_q = np.asarray(inputs["W_q"], np.float32)
    W_k = np.asarray(inputs["W_k"], np.float32)
    W_v = np.asarray(inputs["W_v"], np.float32)
    W_r = np.asarray(inputs["W_r"], np.float32)
    W_p = np.asarray(inputs["W_proj"], np.float32)
    b_q = np.asarray(inputs["b_q"], np.float32)
    b_r = np.asarray(inputs["b_r"], np.float32)
    in_maps = []
    for c in range(N_CORES):
        fq = slice(FQ * c, FQ * (c + 1))
        fr = slice(FR * c, FR * (c + 1))
        br = np.zeros((42, 1), np.float32)
        br[0:NB, 0] = b_r[FR * c:FR * c + NB]
        br[32:32 + NB, 0] = b_r[FR * c + NB:FR * c + 2 * NB]
        in_maps.append({
            "xt": xt,
            "wq": rearr(W_q[:, fq]),
            "wk": rearr(W_k[:, fq]),
            "wv": rearr(W_v[:, fq]),
            "wr": rearr(W_r[:, fr]),
            "wp": np.ascontiguousarray(W_p[fq, :]).astype(BF_NP),
            "bq8": (b_q[fq] * SCALE).reshape(FQ, 1).astype(np.float32),
            "br": br,
            "brev": brev,
        })
    return in_maps


_NC_CACHE = {}
LAST_RESULTS = None


def kernel(**inputs):
    T = np.asarray(inputs["X_bte"]).shape[1]
    if T not in _NC_CACHE:
        _NC_CACHE[T] = build_nc(T)
    nc = _NC_CACHE[T]
    in_maps = make_in_maps(inputs, T)
    from concourse.bass_utils import run_bass_kernel_spmd
    res = run_bass_kernel_spmd(nc, in_maps, core_ids=list(range(N_CORES)))
    global LAST_RESULTS
    LAST_RESULTS = res
    acc = np.zeros((T, E), np.float32)
    for r in res.results:
        acc += r["out_partial"].astype(np.float32)
    acc += np.asarray(inputs["b_proj"], np.float32)[None, :]
    return acc.astype(np.float32)[None]


# revision 47
# speedup vs baseline: 1.0425x; 1.0425x over previous
"""Trainium2 Bass kernel for nn_AttentionLayerBase (relative-position banded attention).

Sharding: 16 heads over 8 cores (2 heads/core, tensor parallel). All matmuls in
bf16 (inputs host-cast), f32 PSUM accumulation. Per core:
  - Q^T, K^T, V (t-major) and R^T projected from X^T; per-head stationaries are
    zero-padded to full K=128 contraction (the HAM clock gate tracks PE *array
    activity*, so partial-height matmuls would pin the clock at 1.2 GHz)
  - per q-tile: P_rev band = R @ reversed(b_nd); the band is EXPONENTIATED
    during its PSUM eviction (exp(S+B) = exp(S) * exp(B)), then a
    diagonal-stride SBUF->SBUF DMA materializes expB[p,j] = exp(P[p, p-j])
    (Toeplitz shear, no HBM round trip); causal mask fills 0 in exp domain
  - S = Q K^T/sqrt(d) -> exp on ACT; one DVE scalar_tensor_tensor fuses the
    exp(S)*expB multiply with the softmax row-sum accumulation
  - A^T tiles via PE transpose (8 per PSUM bank); AV accumulated with
    V-stationary matmuls, both heads col-packed into one PSUM tile
  - output projection per head (zero-padded context stationaries); softmax
    1/rowsum applied per head at PSUM eviction; loop software-pipelined
    (extra(ti+1) prepared during tile ti, outproj(ti) deferred into ti+1)
Partial projections written as bf16; host sums the 8 partials + b_proj in f32.
"""

import numpy as np
import ml_dtypes

import concourse.bass as bass
import concourse.mybir as mybir
import concourse.tile as tile
from concourse import bacc
from concourse.masks import make_identity

FP = mybir.dt.float32
BF = mybir.dt.bfloat16
BF_NP = ml_dtypes.bfloat16
N_HEADS = 16
N_CORES = 8
HPC = N_HEADS // N_CORES  # heads per core = 2
E = 1024
DH = 64            # head dim (qk and v)
FQ = HPC * DH      # per-core q/k/v feature cols = 128
NB = 10            # n_basis
FR = HPC * NB      # per-core r cols = 20
ML = 2048          # max_len of b_nd
SCALE = 1.0 / 8.0  # 1/sqrt(64)
NEG = -1e9


def build_nc(T):
    NT = T // 128
    NE = E // 128
    nc = bacc.Bacc("TRN2", target_bir_lowering=False, debug=False)

    xt_d = nc.dram_tensor("xt", [128, E // 128, T], BF, kind="ExternalInput")
    wq_d = nc.dram_tensor("wq", [128, E // 128, FQ], BF, kind="ExternalInput")
    wk_d = nc.dram_tensor("wk", [128, E // 128, FQ], BF, kind="ExternalInput")
    wv_d = nc.dram_tensor("wv", [128, E // 128, FQ], BF, kind="ExternalInput")
    wr_d = nc.dram_tensor("wr", [128, E // 128, FR], BF, kind="ExternalInput")
    wp_d = nc.dram_tensor("wp", [FQ, E], BF, kind="ExternalInput")
    bq_d = nc.dram_tensor("bq8", [FQ, 1], FP, kind="ExternalInput")
    br_d = nc.dram_tensor("br", [42, 1], FP, kind="ExternalInput")
    brev_d = nc.dram_tensor("brev", [128, ML], BF, kind="ExternalInput")
    out_d = nc.dram_tensor("out_partial", [T, E], BF, kind="ExternalOutput")

    with tile.TileContext(nc) as tc:
        with (
            tc.tile_pool(name="const", bufs=1) as const_pool,
            tc.tile_pool(name="big", bufs=1) as big_pool,
            tc.tile_pool(name="work", bufs=3) as work_pool,
            tc.tile_pool(name="small", bufs=5) as small_pool,
            tc.tile_pool(name="ps_s", bufs=3, space="PSUM") as ps_s,
            tc.tile_pool(name="ps_p", bufs=2, space="PSUM") as ps_p,
            tc.tile_pool(name="ps_t", bufs=2, space="PSUM") as ps_t,
            tc.tile_pool(name="ps_av", bufs=1, space="PSUM") as ps_av,
        ):
            # ---- constants / weights into SBUF ----
            ident = const_pool.tile([128, 128], BF)
            make_identity(nc, ident[:])

            xt_sb = big_pool.tile([128, NE, T], BF)
            wq_sb = const_pool.tile([128, NE, FQ], BF)
            wk_sb = const_pool.tile([128, NE, FQ], BF)
            wv_sb = const_pool.tile([128, NE, FQ], BF)
            wr_sb = const_pool.tile([128, NE, FR], BF)
            wp_sb = const_pool.tile([128, E], BF)
            bq_sb = const_pool.tile([FQ, 1], FP)
            br_sb = const_pool.tile([42, 1], FP)
            brev_sb = const_pool.tile([128, ML], BF)
            # weights first so the first projection matmuls fire as soon as
            # the first X chunks land; X interleaved across both HWDGE rings
            nc.sync.dma_start(wq_sb[:], wq_d[:])
            nc.scalar.dma_start(wk_sb[:], wk_d[:])
            nc.sync.dma_start(wr_sb[:], wr_d[:])
            nc.scalar.dma_start(bq_sb[:], bq_d[:])
            nc.scalar.dma_start(br_sb[:], br_d[:])
            # column-major X load: the projection c0-loop consumes columns
            # in order, so the first matmuls fire after ~1MB instead of 4MB
            for cq in range(0, T, 512):
                for e in range(NE):
                    eng = nc.sync if e % 2 == 0 else nc.scalar
                    eng.dma_start(xt_sb[:, e, cq:cq + 512], xt_d[:, e, cq:cq + 512])
            nc.sync.dma_start(wv_sb[:], wv_d[:])
            nc.scalar.dma_start(brev_sb[:], brev_d[:])
            nc.scalar.dma_start(wp_sb[:], wp_d[:])

            # ---- projections ----
            # Per-head stationaries zero-padded to full K=128 contraction: the
            # HAM clock gate tracks PE *array activity*, so half/partial-height
            # matmuls never register busy enough to unthrottle. Padding rows
            # multiply against the other head's rows times zero.
            qT_pad = [big_pool.tile([128, T], BF, name=f"qTp{h}") for h in range(HPC)]
            kT_sb = big_pool.tile([128, T], BF)
            rT_pad = [big_pool.tile([128, T], BF, name=f"rTp{h}") for h in range(HPC)]
            v_sb = big_pool.tile([128, NT, FQ], BF)    # t-major; head h cols 64h:64h+64
            nc.gpsimd.memset(qT_pad[0][64:128, :], 0.0)
            nc.gpsimd.memset(qT_pad[1][0:64, :], 0.0)
            nc.gpsimd.memset(rT_pad[0][:], 0.0)
            nc.gpsimd.memset(rT_pad[1][:], 0.0)

            for c0 in range(0, T, 512):
                ct = min(c0 + 512, T)
                qp = ps_s.tile([128, 512], FP, name="qp", tag="mm")
                for e in range(NE):
                    nc.tensor.matmul(qp[:, :ct - c0], wq_sb[:, e], xt_sb[:, e, c0:ct],
                                     start=(e == 0), stop=(e == NE - 1))
                nc.scalar.activation(qT_pad[0][0:64, c0:ct], qp[0:64, :ct - c0],
                                     mybir.ActivationFunctionType.Identity,
                                     bias=bq_sb[0:64, 0:1], scale=SCALE)
                nc.scalar.activation(qT_pad[1][64:128, c0:ct], qp[64:128, :ct - c0],
                                     mybir.ActivationFunctionType.Identity,
                                     bias=bq_sb[64:128, 0:1], scale=SCALE)
                kp = ps_s.tile([128, 512], FP, name="kp", tag="mm")
                for e in range(NE):
                    nc.tensor.matmul(kp[:, :ct - c0], wk_sb[:, e], xt_sb[:, e, c0:ct],
                                     start=(e == 0), stop=(e == NE - 1))
                nc.vector.tensor_copy(kT_sb[:, c0:ct], kp[:, :ct - c0])
                # R^T both heads in one psum tile, col groups 0 / 1
                rp = ps_p.tile([128, 512], FP, name="rp", tag="pp")
                for e in range(NE):
                    nc.tensor.matmul(rp[0:NB, :ct - c0], wr_sb[:, e, 0:NB],
                                     xt_sb[:, e, c0:ct],
                                     start=(e == 0), stop=(e == NE - 1))
                    nc.tensor.matmul(rp[32:32 + NB, :ct - c0], wr_sb[:, e, NB:2 * NB],
                                     xt_sb[:, e, c0:ct],
                                     start=(e == 0), stop=(e == NE - 1))
                nc.scalar.activation(rT_pad[0][0:NB, c0:ct], rp[0:NB, :ct - c0],
                                     mybir.ActivationFunctionType.Identity,
                                     bias=br_sb[0:NB, 0:1], scale=1.0)
                nc.scalar.activation(rT_pad[1][32:32 + NB, c0:ct], rp[32:32 + NB, :ct - c0],
                                     mybir.ActivationFunctionType.Identity,
                                     bias=br_sb[32:32 + NB, 0:1], scale=1.0)
            for ti in range(NT):
                vp = ps_s.tile([128, 512], FP, name="vp", tag="mm")[:, :FQ]
                for e in range(NE):
                    nc.tensor.matmul(vp[:], xt_sb[:, e, ti * 128:(ti + 1) * 128],
                                     wv_sb[:, e], start=(e == 0), stop=(e == NE - 1))
                nc.vector.tensor_copy(v_sb[:, ti], vp[:])

            # ---- attention: software-pipelined over q-tiles ----
            # prep(ti): P_rev band matmuls -> bf16 evict -> diagonal-shear DMA
            # -> causal mask, producing extra tiles for tile ti.
            def prep(ti):
                W = 128 * (ti + 1)
                pbands = [work_pool.tile([128, T + 128], BF, name=f"pband{h}")
                          for h in range(HPC)]
                for cs in range(0, W, 512):
                    ce = min(cs + 512, W)
                    pps = []
                    for h in range(HPC):
                        pp = ps_p.tile([128, 512], FP, name="pp", tag="pp")
                        nc.tensor.matmul(pp[:, :ce - cs],
                                         rT_pad[h][:, ti * 128:(ti + 1) * 128],
                                         brev_sb[:, ML - W + cs:ML - W + ce],
                                         start=True, stop=True)
                        pps.append(pp)
                    for h in range(HPC):
                        # exp(S + extra) = exp(S) * exp(extra): exponentiate the
                        # band during its PSUM eviction (same-cost ACT pass)
                        nc.scalar.activation(pbands[h][:, cs:ce], pps[h][:, :ce - cs],
                                             mybir.ActivationFunctionType.Exp)
                extras = []
                for h in range(HPC):
                    extra = work_pool.tile([128, T], BF, name=f"extra{h}")
                    # shear: extra[p, j] = pband[p, 127 - p + j]; stride is
                    # relative to the tile's flat row length (T + 128)
                    esrc = bass.AP(pbands[h][:].tensor, 127, [[T + 128 - 1, 128], [1, W]])
                    nc.sync.dma_start(extra[:, :W], esrc)
                    nc.gpsimd.affine_select(
                        out=extra[:, W - 128:W], in_=extra[:, W - 128:W],
                        compare_op=mybir.AluOpType.is_ge, fill=0.0,
                        base=0, channel_multiplier=1, pattern=[[-1, 128]])
                    extras.append(extra)
                return extras

            def outproj(ti, recips, outps):
                po_sb = work_pool.tile([128, E], BF, name="po")
                for n0 in range(0, E, 512):
                    ops = []
                    for h in range(HPC):
                        op = ps_s.tile([128, 512], FP, name=f"op{h}", tag="mm")
                        nc.tensor.matmul(op[:], outps[h][:],
                                         wp_sb[:, n0:n0 + 512],
                                         start=True, stop=True)
                        ops.append(op)
                    if n0 == 0:
                        nc.scalar.mul(po_sb[:, n0:n0 + 512], ops[0][:], recips[0][:, 0:1])
                    else:
                        nc.vector.tensor_scalar_mul(po_sb[:, n0:n0 + 512], ops[0][:],
                                                    recips[0][:, 0:1])
                    nc.vector.scalar_tensor_tensor(
                        po_sb[:, n0:n0 + 512], ops[1][:], recips[1][:, 0:1],
                        po_sb[:, n0:n0 + 512],
                        mybir.AluOpType.mult, mybir.AluOpType.add)
                nc.sync.dma_start(out_d[ti * 128:(ti + 1) * 128, :], po_sb[:])

            extras = prep(0)
            prev = None  # (ti, recips) pending output projection
            for ti in range(NT):
                W = 128 * (ti + 1)
                # S = Q K^T/8 + extra, exp with fused row sums
                aexps = [work_pool.tile([128, T], BF, name=f"aexp{h}")
                         for h in range(HPC)]
                aexpSs = [work_pool.tile([128, T], BF, name=f"aexpS{h}")
                          for h in range(HPC)]
                sums4 = [small_pool.tile([128, T // 512 + 1], FP, name=f"sums4{h}")
                         for h in range(HPC)]
                nch = 0
                for cs in range(0, W, 512):
                    ce = min(cs + 512, W)
                    sps = []
                    for h in range(HPC):
                        sp = ps_s.tile([128, 512], FP, name="sp", tag="mm")
                        nc.tensor.matmul(sp[:, :ce - cs],
                                         qT_pad[h][:, ti * 128:(ti + 1) * 128],
                                         kT_sb[:, cs:ce],
                                         start=True, stop=True)
                        sps.append(sp)
                    for h in range(HPC):
                        nc.scalar.activation(aexpSs[h][:, cs:ce], sps[h][:, :ce - cs],
                                             mybir.ActivationFunctionType.Exp)
                        nc.vector.scalar_tensor_tensor(
                            aexps[h][:, cs:ce], aexpSs[h][:, cs:ce], 1.0,
                            extras[h][:, cs:ce],
                            mybir.AluOpType.bypass, mybir.AluOpType.mult,
                            accum_out=sums4[h][:, nch:nch + 1])
                    nch += 1

                # deferred output projection of the previous tile (its avp
                # eviction is long done -> no PE FIFO stall)
                if prev is not None:
                    outproj(*prev)

                # prepare extra for the NEXT tile while exp/AV of this one runs
                if ti + 1 < NT:
                    next_extras = prep(ti + 1)

                recips = []
                for h in range(HPC):
                    sums1 = small_pool.tile([128, 1], FP, name="sums1")
                    if nch > 1:
                        nc.vector.tensor_reduce(sums1[:], sums4[h][:, :nch],
                                                axis=mybir.AxisListType.X,
                                                op=mybir.AluOpType.add)
                    else:
                        nc.vector.tensor_copy(sums1[:], sums4[h][:, 0:1])
                    recip = small_pool.tile([128, 1], FP, name=f"recip{h}")
                    nc.vector.reciprocal(recip[:], sums1[:])
                    recips.append(recip)

                # A^T tiles (batched transposes) + AV accumulate; both heads
                # col-packed into one PSUM tile [128 (h,d), 128 q]
                avp = ps_av.tile([128, 128], FP, name="avp")
                for j0 in range(0, ti + 1, 8):
                    j1 = min(j0 + 8, ti + 1)
                    aT4s = []
                    for h in range(HPC):
                        tp4 = ps_t.tile([128, 1024], BF, name="tp4")
                        for c in range(j1 - j0):
                            j = j0 + c
                            nc.tensor.matmul(tp4[:, c * 128:(c + 1) * 128],
                                             aexps[h][:, j * 128:(j + 1) * 128],
                                             ident[:], is_transpose=True)
                        aT4 = small_pool.tile([128, 1024], BF, name="aT4")
                        nc.vector.tensor_copy(aT4[:, :(j1 - j0) * 128],
                                              tp4[:, :(j1 - j0) * 128])
                        aT4s.append(aT4)
                    for c in range(j1 - j0):
                        j = j0 + c
                        for h in range(HPC):
                            nc.tensor.matmul(avp[64 * h:64 * h + DH, :],
                                             v_sb[:, j, DH * h:DH * (h + 1)],
                                             aT4s[h][:, c * 128:(c + 1) * 128],
                                             start=(j == 0), stop=(j == ti))
                outps = [small_pool.tile([128, 128], BF, name=f"outp{h}")
                         for h in range(HPC)]
                nc.gpsimd.memset(outps[0][64:128, :], 0.0)
                nc.gpsimd.memset(outps[1][0:64, :], 0.0)
                nc.vector.tensor_copy(outps[0][0:64, :], avp[0:64, :])
                nc.vector.tensor_copy(outps[1][64:128, :], avp[64:128, :])
                prev = (ti, recips, outps)
                if ti + 1 < NT:
                    extras = next_extras
            outproj(*prev)

    nc.compile()
    return nc


def make_in_maps(inputs, T):
    X = np.asarray(inputs["X_bte"], np.float32)[0]  # (T, E)
    xt = np.ascontiguousarray(
        X.T.reshape(E // 128, 128, -1).transpose(1, 0, 2)).astype(BF_NP)
    rearr = lambda w: np.ascontiguousarray(
        w.reshape(E // 128, 128, -1).transpose(1, 0, 2)).astype(BF_NP)
    brev_1 = np.asarray(inputs["b_nd"], np.float32)[:, ::-1].astype(BF_NP)
    brev = np.zeros((128, ML), BF_NP)
    brev[0:NB] = brev_1
    brev[32:32 + NB] = brev_1
    W_q = np.asarray(inputs["W_q"], np.float32)
    W_k = np.asarray(inputs["W_k"], np.float32)
    W_v = np.asarray(inputs["W_v"], np.float32)
    W_r = np.asarray(inputs["W_r"], np.float32)
    W_p = np.asarray(inputs["W_proj"], np.float32)
    b_q = np.asarray(inputs["b_q"], np.float32)
    b_r = np.asarray(inputs["b_r"], np.float32)
    in_maps = []
    for c in range(N_CORES):
        fq = slice(FQ * c, FQ * (c + 1))
        fr = slice(FR * c, FR * (c + 1))
        br = np.zeros((42, 1), np.float32)
        br[0:NB, 0] = b_r[FR * c:FR * c + NB]
        br[32:32 + NB, 0] = b_r[FR * c + NB:FR * c + 2 * NB]
        in_maps.append({
            "xt": xt,
            "wq": rearr(W_q[:, fq]),
            "wk": rearr(W_k[:, fq]),
            "wv": rearr(W_v[:, fq]),
            "wr": rearr(W_r[:, fr]),
            "wp": np.ascontiguousarray(W_p[fq, :]).astype(BF_NP),
            "bq8": (b_q[fq] * SCALE).reshape(FQ, 1).astype(np.float32),
            "br": br,
            "brev": brev,
        })
    return in_maps


_NC_CACHE = {}
LAST_RESULTS = None


def kernel(**inputs):
    T = np.asarray(inputs["X_bte"]).shape[1]
    if T not in _NC_CACHE:
        _NC_CACHE[T] = build_nc(T)
    nc = _NC_CACHE[T]
    in_maps = make_in_maps(inputs, T)
    from concourse.bass_utils import run_bass_kernel_spmd
    res = run_bass_kernel_spmd(nc, in_maps, core_ids=list(range(N_CORES)))
    global LAST_RESULTS
    LAST_RESULTS = res
    acc = np.zeros((T, E), np.float32)
    for r in res.results:
        acc += r["out_partial"].astype(np.float32)
    acc += np.asarray(inputs["b_proj"], np.float32)[None, :]
    return acc.astype(np.float32)[None]
